# revision 23
# baseline (speedup 1.0000x reference)
"""Trainium2 Bass kernel for nn_Attention_Block (dense transformer block).

Strategy: pure data-parallel over batch — 8 samples, 8 NeuronCores, one
sample per core, weights replicated, no collectives. Per core everything
stays channels-on-partitions (c x n layout):

  GN1 (bn_stats + selector-matmul group reduce) -> QKV matmul (bf16) ->
  per-head attention with row-packed score matmuls (even head on PE rows
  0:64, odd head on rows 64:128 via tile_position, running concurrently),
  V^T computed directly as xn^T @ Wv (no PE transposes), softmax via exp
  + ones-column denominator in the AV matmul -> out-proj (V bias folded
  in on host) -> GN2 -> SwiGLU MLP -> +residual.

Matmuls run in bf16 (full PE rate); stats/softmax denominators in fp32.
"""

import os

import numpy as np
import ml_dtypes

KSTAGE = int(os.environ.get("KSTAGE", "7"))

C = 512
NSP = 1024  # 32*32 spatial
CT = 4  # channel tiles of 128
HEADS = 8
D = 64
HID = 2048
EPS = 1e-5
FS = 16.0  # fp8 weight scale for the MLP (weights are tiny; scale into
# e4m3's normal range, unscale via ACT's free input scale / scalar.mul)

_cache = {}


def _patch_tile_drain(tile, mybir):
    """walrus in this environment accepts very few sync waits per
    instruction; the TileContext tail drain carries one wait per proc of
    the global clock. Split them across preceding SP drains."""
    if getattr(tile.TileContext, "_drain_patched", False):
        return

    def _patched(self, tick_clock, wait_clock):
        nc = self.nc
        spills = [nc.sync.drain() for _ in range(40)]
        drain_inst = nc.sync.drain()
        wait_clock.add_sem_waits(
            drain_inst.ins, tile.ScopedClock({None: tick_clock.global_clock})
        )
        si = drain_inst.ins.sync_info
        waits = list(si.on_wait) if si is not None and si.on_wait else []
        upds = list(si.on_update) if si is not None and si.on_update else []
        if len(waits) > 1:
            *pre, last = waits
            assert len(pre) <= len(spills), "too many drain wait chunks"
            for sp_inst, w in zip(spills, pre):
                sp_inst.ins.sync_info = mybir.SyncInfo(on_wait=[w], on_update=[])
            drain_inst.ins.sync_info = mybir.SyncInfo(on_wait=[last], on_update=upds)
        nc.all_engine_barrier()
        assert self.sems is not None
        popped = nc._tile_sem_poison_stack.pop()
        assert popped is self._sem_poison
        nc.clear_and_free_semaphores(list(self.sems.allocated().values()))
        nc.all_engine_barrier()

    tile.TileContext._drain_and_barrier = _patched
    tile.TileContext._drain_patched = True


def _split_multi_waits(nc, mybir, maxw=1):
    """Hoist extra sync waits onto same-engine EventSemaphore carriers so
    no instruction carries more than `maxw` waits."""
    f = nc.m.functions[0]
    for bb in f.blocks:
        insts = list(bb.instructions)
        need = [
            i
            for i in insts
            if getattr(i, "sync_info", None)
            and i.sync_info.on_wait
            and len(i.sync_info.on_wait) > maxw
        ]
        if not need:
            continue
        carriers = {}
        for inst in need:
            w = list(inst.sync_info.on_wait)
            upds = list(inst.sync_info.on_update) if inst.sync_info.on_update else []
            keep = w[-maxw:]
            extra = w[:-maxw]
            cs = []
            for i in range(0, len(extra), maxw):
                c = mybir.InstEventSemaphore(
                    name=f"I-waitc-{nc.next_id()}", ins=[], outs=[]
                )
                c.engine = inst.engine
                c.sync_info = mybir.SyncInfo(on_wait=extra[i : i + maxw], on_update=[])
                nc.register_instruction(c)
                cs.append(c)
            inst.sync_info = mybir.SyncInfo(on_wait=keep, on_update=upds)
            carriers[inst.name] = cs
        carrier_names = {c.name for cs in carriers.values() for c in cs}
        rebuilt = []
        for inst in list(bb.instructions):
            if inst.name in carrier_names:
                continue
            if inst.name in carriers:
                rebuilt.extend(carriers[inst.name])
            rebuilt.append(inst)
        bb.instructions = rebuilt


def _build_nc():
    import concourse.bass as bass
    import concourse.tile as tile
    from concourse import mybir

    _patch_tile_drain(tile, mybir)

    F32 = mybir.dt.float32
    BF16 = mybir.dt.bfloat16
    FP8 = mybir.dt.float8e4
    DRSWI = mybir.MatmulPerfMode.DoubleRowSwInterleave
    ADD = mybir.AluOpType.add
    SUB = mybir.AluOpType.subtract
    MULT = mybir.AluOpType.mult
    AF = mybir.ActivationFunctionType

    nc = bass.Bass()

    x_d = nc.declare_dram_parameter("x", [C, NSP], F32, isOutput=False)
    wqk8_d = nc.declare_dram_parameter("wqk8", [128, 2 * 8 * 256], FP8, isOutput=False)
    wv8_d = nc.declare_dram_parameter("wv8", [128, 4 * 512], FP8, isOutput=False)
    qkvb_d = nc.declare_dram_parameter("qkvb", [128, 8], F32, isOutput=False)
    wo8_d = nc.declare_dram_parameter("wo8", [128, 2 * 4 * 256], FP8, isOutput=False)
    outb_d = nc.declare_dram_parameter("outb", [128, 4], F32, isOutput=False)
    g1_d = nc.declare_dram_parameter("g1", [128, 4], F32, isOutput=False)
    b1_d = nc.declare_dram_parameter("b1", [128, 4], F32, isOutput=False)
    g2_d = nc.declare_dram_parameter("g2", [128, 4], F32, isOutput=False)
    b2_d = nc.declare_dram_parameter("b2", [128, 4], F32, isOutput=False)
    w1_d = nc.declare_dram_parameter("w1f8", [128, 2 * 32 * 256], FP8, isOutput=False)
    w2_d = nc.declare_dram_parameter("w2f8", [128, 8 * 4 * 256], FP8, isOutput=False)
    sel_d = nc.declare_dram_parameter("sel", [C, 32], F32, isOutput=False)
    selT_d = nc.declare_dram_parameter("selT", [32, C], F32, isOutput=False)
    id_d = nc.declare_dram_parameter("ident", [128, 128], BF16, isOutput=False)
    selbc8_d = nc.declare_dram_parameter("selbc8", [8, 512], BF16, isOutput=False)
    out_d = nc.declare_dram_parameter("out", [C, NSP], F32, isOutput=True)

    with tile.TileContext(nc) as tc:
        with (
            tc.tile_pool(name="pers", bufs=1) as pers,
            tc.tile_pool(name="gnp", bufs=2) as gnp,
            tc.tile_pool(name="expp", bufs=20) as expp,
            tc.tile_pool(name="swp", bufs=2) as swp,
            tc.tile_pool(name="unp", bufs=4) as unp,
            tc.tile_pool(name="invp", bufs=2) as invp,
            tc.tile_pool(name="psb", bufs=3, space="PSUM") as psb_pool,
            tc.tile_pool(name="pss", bufs=2, space="PSUM") as pss_pool,
        ):
            def bigps():
                return psb_pool.tile([128, NSP], F32, tag="psb", name="psb")

            def smallps(shape, dtype):
                return pss_pool.tile(shape, dtype, tag="pss", name="pss")

            # ---- input loads (x lands in the attn2 slots; reloaded later) ----
            x_sb = []
            for t in range(CT):
                xt = pers.tile([128, NSP], F32, tag=f"attn2{t}", name=f"attn2{t}")
                # halves so bn_stats can chase the DMA
                for h in range(2):
                    s = slice(h * 512, (h + 1) * 512)
                    nc.sync.dma_start(xt[:, s], x_d[t * 128 : (t + 1) * 128, s])
                x_sb.append(xt)
            sel_sb = []
            for t in range(CT):
                st = pers.tile([128, 32], F32, tag=f"sel{t}", name=f"sel{t}")
                nc.sync.dma_start(st[:], sel_d[t * 128 : (t + 1) * 128, :])
                sel_sb.append(st)
            selT_sb = pers.tile([32, C], F32, tag="selT", name="selT")
            nc.sync.dma_start(selT_sb[:], selT_d[:, :])
            g1_sb = pers.tile([128, 4], F32, tag="g1", name="g1")
            nc.sync.dma_start(g1_sb[:], g1_d[:, :])
            b1_sb = pers.tile([128, 4], F32, tag="b1", name="b1")
            nc.sync.dma_start(b1_sb[:], b1_d[:, :])
            wqk8_sb = pers.tile([128, 2 * 8 * 256], FP8, tag="wqk8", name="wqk8")
            nc.sync.dma_start(wqk8_sb[:], wqk8_d[:, :])
            wv8_sb = pers.tile([128, 4, 512], FP8, tag="wv8", name="wv8")
            nc.sync.dma_start(
                wv8_sb[:].rearrange("p a b -> p (a b)"), wv8_d[:, :]
            )

            def wqk8_view(P, mt):
                o = (P * 8 + mt) * 256
                return wqk8_sb[:, o : o + 256].rearrange("p (a b) -> p a b", a=2)
            qkvb_sb = pers.tile([128, 8], F32, tag="qkvb", name="qkvb")
            nc.sync.dma_start(qkvb_sb[:], qkvb_d[:, :])
            selbc8_sb = pers.tile([8, 512], BF16, tag="selbc8", name="selbc8")
            nc.sync.dma_start(selbc8_sb[:], selbc8_d[:, :])
            id_sb = pers.tile([128, 128], BF16, tag="ident", name="ident")
            nc.sync.dma_start(id_sb[:], id_d[:, :])
            wo8_sb = pers.tile([128, 2 * 4 * 256], FP8, tag="wo8", name="wo8")
            nc.sync.dma_start(wo8_sb[:], wo8_d[:, :])

            def wo8_view(P, mt):
                o = (P * 4 + mt) * 256
                return wo8_sb[:, o : o + 256].rearrange("p (a b) -> p a b", a=2)
            outb_sb = pers.tile([128, 4], F32, tag="outb", name="outb")
            nc.sync.dma_start(outb_sb[:], outb_d[:, :])
            g2_sb = pers.tile([128, 4], F32, tag="g2", name="g2")
            nc.sync.dma_start(g2_sb[:], g2_d[:, :])
            b2_sb = pers.tile([128, 4], F32, tag="b2", name="b2")
            nc.sync.dma_start(b2_sb[:], b2_d[:, :])
            w1_sb = pers.tile([128, 2 * 32 * 256], FP8, tag="w1f8", name="w1f8")
            nc.sync.dma_start(w1_sb[:], w1_d[:, :])
            w2_sb = pers.tile([128, 8 * 4 * 256], FP8, tag="w2f8", name="w2f8")
            nc.sync.dma_start(w2_sb[:], w2_d[:, :])

            def w1_view(P, mt):  # interleaved DR weight block [128, 2, 128]
                o = (P * 32 + mt) * 256
                return w1_sb[:, o : o + 256].rearrange("p (a b) -> p a b", a=2)

            def w2_view(P, mt):
                o = (P * 4 + mt) * 256
                return w2_sb[:, o : o + 256].rearrange("p (a b) -> p a b", a=2)

            eps32 = pers.tile([32, 1], F32, tag="eps", name="eps")
            nc.vector.memset(eps32[:], EPS)
            ones65 = pers.tile([65, 1], F32, tag="ones65", name="ones65")
            nc.vector.memset(ones65[:], 1.0)
            actdum = pers.tile([32, 1], F32, tag="actdum", name="actdum")

            def act_prefetch(func):
                # dummy activation to pull the ACT table-set load off the
                # critical path (set switches cost ~2.7us)
                nc.scalar.activation(actdum[:], eps32[:], func)

            act_prefetch(AF.Sqrt)

            # ---- group norm helper (32 groups of 16 channels x 1024) ----
            def group_norm(src_tiles, gam_sb, bet_sb, dst_tiles):
                rhs3 = []
                for t in range(CT):
                    stats = gnp.tile([128, 2, 6], F32, tag="gn_stats", name="gn_stats")
                    for j2 in range(2):
                        nc.vector.bn_stats(
                            stats[:, j2, :], src_tiles[t][:, j2 * 512 : (j2 + 1) * 512]
                        )
                    mv = gnp.tile([128, 2], F32, tag="gn_mv", name="gn_mv")
                    nc.vector.bn_aggr(mv[:], stats[:])
                    r3 = gnp.tile([128, 3], F32, tag=f"gn_r3_{t}", name=f"gn_r3_{t}")
                    nc.vector.tensor_copy(r3[:, 0:2], mv[:])
                    nc.vector.tensor_mul(r3[:, 2:3], mv[:, 0:1], mv[:, 0:1])
                    rhs3.append(r3)
                pg = smallps([32, 3], F32)
                for t in range(CT):
                    nc.tensor.matmul(
                        pg[:], sel_sb[t][:], rhs3[t][:], start=(t == 0), stop=(t == 3)
                    )
                gs = gnp.tile([32, 2], F32, tag="gn_gs", name="gn_gs")
                tmp = gnp.tile([32, 2], F32, tag="gn_tmp", name="gn_tmp")
                pgs = gnp.tile([32, 3], F32, tag="gn_pgs", name="gn_pgs")
                nc.vector.tensor_copy(pgs[:], pg[:])
                # mean_g, E[x^2]_g, var_g, rstd_g
                nc.vector.tensor_scalar_mul(gs[:, 0:1], pgs[:, 0:1], 1.0 / 16)
                nc.vector.tensor_tensor(tmp[:, 0:1], pgs[:, 1:2], pgs[:, 2:3], op=ADD)
                nc.vector.tensor_scalar_mul(tmp[:, 0:1], tmp[:, 0:1], 1.0 / 16)
                nc.vector.tensor_mul(tmp[:, 1:2], gs[:, 0:1], gs[:, 0:1])
                nc.vector.tensor_tensor(tmp[:, 0:1], tmp[:, 0:1], tmp[:, 1:2], op=SUB)
                nc.scalar.activation(
                    tmp[:, 0:1], tmp[:, 0:1], AF.Sqrt, bias=eps32[:]
                )
                nc.vector.reciprocal(gs[:, 1:2], tmp[:, 0:1])
                for t in range(CT):
                    pbc = smallps([128, 2], F32)
                    nc.tensor.matmul(
                        pbc[:],
                        selT_sb[:, t * 128 : (t + 1) * 128],
                        gs[:],
                        start=True,
                        stop=True,
                    )
                    a_t = gnp.tile([128, 1], F32, tag="gn_A", name="gn_A")
                    b_t = gnp.tile([128, 1], F32, tag="gn_B", name="gn_B")
                    nc.vector.tensor_mul(a_t[:], pbc[:, 1:2], gam_sb[:, t : t + 1])
                    nc.vector.tensor_mul(b_t[:], pbc[:, 0:1], a_t[:])
                    nc.vector.tensor_tensor(
                        b_t[:], bet_sb[:, t : t + 1], b_t[:], op=SUB
                    )
                    nc.vector.tensor_scalar(
                        dst_tiles[t],
                        src_tiles[t][:],
                        scalar1=a_t[:],
                        scalar2=b_t[:],
                        op0=MULT,
                        op1=ADD,
                    )

            # ---- GN1 -> xn8a (fp8) ----
            xn8a = pers.tile([128, CT, NSP], FP8, tag="xn8a", name="xn8a")
            group_norm(x_sb, g1_sb, b1_sb, [xn8a[:, t, :] for t in range(CT)])
            act_prefetch(AF.Exp)

            def dump_and_finish(tiles, cast=True, reuse=None):
                for t in range(CT):
                    if cast:
                        if reuse is not None:
                            ft = reuse[t]
                        else:
                            ft = pers.tile(
                                [128, NSP], F32, tag=f"dump{t}", name=f"dump{t}"
                            )
                        nc.vector.tensor_copy(ft[:], tiles[t])
                        nc.sync.dma_start(out_d[t * 128 : (t + 1) * 128, :], ft[:])
                    else:
                        nc.sync.dma_start(out_d[t * 128 : (t + 1) * 128, :], tiles[t])

            if KSTAGE == 1:
                dump_and_finish([xn8a[:, t, :] for t in range(CT)])
                return nc, tc

            # ---- QKV q/k tiles (8 of 128 x 1024, bf16, bias added) ----
            # m 0..3 = q tiles, 4..7 = k tiles. Emit j=0's pair first so
            # attention scores (feeding the ACT exp bottleneck) start early.
            qkv = [None] * 8

            def qkv_tile(m):
                qkv[m] = pers.tile([128, NSP], BF16, tag=f"qkv{m}", name=f"qkv{m}")
                ps = bigps()
                for n2 in range(2):
                    s = slice(n2 * 512, (n2 + 1) * 512)
                    for P in range(2):
                        nc.tensor.matmul(
                            ps[:, s],
                            wqk8_view(P, m),
                            xn8a[:, 2 * P : 2 * P + 2, s],
                            start=(P == 0),
                            stop=(P == 1),
                            perf_mode=DRSWI,
                        )
                nc.vector.tensor_scalar(
                    qkv[m][:], ps[:],
                    scalar1=1.0 / FS, scalar2=qkvb_sb[:, m : m + 1],
                    op0=MULT, op1=ADD,
                )

            qkv_tile(0)
            qkv_tile(4)

            # ---- per-j score + exp emission (zero-padded K tiles: kpe has
            # the even head's K rows 0:64 and zeros below, kpo vice versa;
            # full-128 contraction keeps the PE in its fast path; exp lands
            # as fp8 key-chunk PAIRS for the DoubleRow AV) ----
            kpe = pers.tile([128, NSP], BF16, tag="kpe", name="kpe")
            kpo = pers.tile([128, NSP], BF16, tag="kpo", name="kpo")
            nc.vector.memset(kpe[64:128, :], 0.0)
            nc.vector.memset(kpo[0:64, :], 0.0)
            kp = [kpe, kpo]
            exps = {}  # (j, side, P) -> [128, 2, 1024] fp8 exp tile

            def scores_exp(j):
                nc.vector.tensor_copy(kpe[0:64, :], qkv[4 + j][0:64, :])
                nc.vector.tensor_copy(kpo[64:128, :], qkv[4 + j][64:128, :])
                for mk in range(8):
                    P, par = mk // 2, mk % 2
                    mks = slice(mk * 128, (mk + 1) * 128)
                    sc = [bigps(), bigps()]
                    for n2 in range(2):
                        s = slice(n2 * 512, (n2 + 1) * 512)
                        for side in range(2):
                            nc.tensor.matmul(
                                sc[side][:, s],
                                kp[side][:, mks],
                                qkv[j][:, s],
                                start=True,
                                stop=True,
                            )
                    for side in range(2):
                        if par == 0:
                            exps[(j, side, P)] = expp.tile(
                                [128, 2, NSP], FP8, tag="exp", name="exp"
                            )
                        nc.scalar.activation(
                            exps[(j, side, P)][:, par, :], sc[side][:],
                            AF.Exp, scale=0.125,
                        )

            scores_exp(0)

            # ---- V^T via matmul, stored DoubleRowSwInterleave-ready:
            # vt8 blocks of 256 per (key-chunk pair P, head js): logical
            # weight cols = [V(64) | ones | 63 zeros], interleaved A/B
            # chunk-pair and column-reversed ----
            vt8 = pers.tile([128, 4 * 8 * 256], FP8, tag="vt8", name="vt8")
            vt8v = vt8[:].rearrange("p (P j t i) -> p P j t i", P=4, j=8, i=2)
            nc.vector.memset(vt8[:], 0.0)
            nc.vector.memset(vt8v[:, :, :, 63, :], 1.0)

            def vt8_lhs(P, js):
                o = (P * 8 + js) * 256
                return vt8[:, o : o + 256].rearrange("p (a b) -> p a b", a=2)

            # xn interleaved for the V^T DR lhsT: block (P, mk) of 256:
            # flat[2t+i] = xn8a[p, 2P+i, mk*128 + (127-t)]
            xn8i = pers.tile([128, 2 * 8 * 256], FP8, tag="xn8i", name="xn8i")
            xn8iv = xn8i[:].rearrange("p (P m t i) -> p P m t i", P=2, m=8, i=2)
            for P in range(2):
                for i in range(2):
                    src = xn8a[:, 2 * P + i, :].rearrange(
                        "p (m d) -> p m d", m=8
                    )
                    nc.vector.tensor_copy(
                        xn8iv[:, P, :, :, i], src[:, :, ::-1]
                    )

            def xn8i_lhs(P, mk):
                o = (P * 8 + mk) * 256
                return xn8i[:, o : o + 256].rearrange("p (a b) -> p a b", a=2)

            for mk in range(8):
                P, par = mk // 2, mk % 2
                vps = smallps([128, 512], F32)
                for kP in range(2):
                    nc.tensor.matmul(
                        vps[:],
                        xn8i_lhs(kP, mk),
                        wv8_sb[:, 2 * kP : 2 * kP + 2, :],
                        start=(kP == 0),
                        stop=(kP == 1),
                        perf_mode=DRSWI,
                    )
                src = vps[:].rearrange("p (h d) -> p h d", h=8)
                nc.vector.tensor_copy(
                    vt8v[:, P, :, 64:128, par], src[:, :, ::-1]
                )

            for m in (1, 5, 2, 6, 3, 7):
                qkv_tile(m)

            if KSTAGE == 2:
                dump_and_finish([t[:] for t in qkv[0:4]])
                return nc, tc

            # ---- attention: AV + softmax normalize per j, 512-query phases
            # (xattn lands fp8 at 16x scale: V^T carried the FS factor) ----
            xattn8 = pers.tile([128, 2, 2, NSP], FP8, tag="xattn8", name="xattn8")
            for j in range(4):
                if j < 3:
                    scores_exp(j + 1)
                for ph in range(2):
                    phs = slice(ph * 512, (ph + 1) * 512)
                    uns = []
                    for side in range(2):
                        pav = smallps([128, 512], F32)
                        for P in range(4):
                            nc.tensor.matmul(
                                pav[:],
                                vt8_lhs(P, 2 * j + side),
                                exps[(j, side, P)][:, :, phs],
                                start=(P == 0),
                                stop=(P == 3),
                                perf_mode=DRSWI,
                            )
                        un = unp.tile([65, 512], F32, tag="un", name="un")
                        nc.vector.tensor_copy(un[:], pav[0:65, :])
                        uns.append(un)
                    # denominators: transpose the 8 denom chunks into
                    # partitions, one parallel reciprocal, transpose back,
                    # selector-matmul broadcast over 64 partitions.
                    pdt = smallps([128, 8], F32)
                    for side in range(2):
                        for jj in range(4):
                            c = side * 4 + jj
                            nc.tensor.transpose(
                                pdt[:, c : c + 1],
                                uns[side][64:65, jj * 128 : (jj + 1) * 128],
                                ones65[64:65, 0:1],
                            )
                    inv8 = invp.tile([128, 8], F32, tag="inv", name="inv")
                    nc.vector.reciprocal(inv8[:], pdt[:])
                    inv8b = invp.tile([128, 8], BF16, tag="invb", name="invb")
                    nc.vector.tensor_copy(inv8b[:], inv8[:])
                    ptv = smallps([8, 128], BF16)
                    nc.tensor.transpose(ptv[:], inv8b[:], id_sb[:])
                    pts = invp.tile([8, 128], BF16, tag="pts", name="pts")
                    nc.vector.tensor_copy(pts[:], ptv[:])
                    for side in range(2):
                        pinvb = smallps([64, 512], F32)
                        for jj in range(4):
                            r = side * 4 + jj
                            nc.tensor.matmul(
                                pinvb[:, jj * 128 : (jj + 1) * 128],
                                selbc8_sb[:, r * 64 : (r + 1) * 64],
                                pts[:],
                                start=True,
                                stop=True,
                            )
                        nc.vector.tensor_mul(
                            xattn8[64 * side : 64 * side + 64, j // 2, j % 2, phs],
                            uns[side][0:64, :],
                            pinvb[0:64, :],
                        )

            if KSTAGE == 3:
                for t in range(CT):
                    ft = pers.tile([128, NSP], F32, tag=f"dump{t}", name=f"dump{t}")
                    nc.scalar.mul(ft[:], xattn8[:, t // 2, t % 2, :], 1.0 / FS)
                    nc.sync.dma_start(out_d[t * 128 : (t + 1) * 128, :], ft[:])
                return nc, tc

            # ---- out projection (keep f32 for GN2 stats) ----
            attn2 = [
                pers.tile([128, NSP], F32, tag=f"attn2{t}", name=f"attn2{t}")
                for t in range(CT)
            ]
            for m in range(CT):
                ps = bigps()
                for n2 in range(2):
                    s = slice(n2 * 512, (n2 + 1) * 512)
                    for P in range(2):
                        nc.tensor.matmul(
                            ps[:, s],
                            wo8_view(P, m),
                            xattn8[:, P, :, s],
                            start=(P == 0),
                            stop=(P == 1),
                            perf_mode=DRSWI,
                        )
                nc.vector.tensor_scalar(
                    attn2[m][:], ps[:],
                    scalar1=1.0 / (FS * FS), scalar2=outb_sb[:, m : m + 1],
                    op0=MULT, op1=ADD,
                )

            if KSTAGE == 4:
                dump_and_finish([t[:] for t in attn2], cast=False)
                return nc, tc

            # ---- GN2 -> xn8 (fp8 for the DoubleRow MLP) ----
            xn8 = pers.tile([128, CT, NSP], FP8, tag="xn8", name="xn8")
            group_norm(attn2, g2_sb, b2_sb, [xn8[:, t, :] for t in range(CT)])
            act_prefetch(AF.Silu)

            if KSTAGE == 5:
                dump_and_finish([xn8[:, t, :] for t in range(CT)], reuse=attn2)
                return nc, tc

            # ---- MLP1 + SwiGLU -> act8 (fp8, scaled by FS) ----
            act8 = pers.tile([128, 16, NSP], FP8, tag="act8", name="act8")
            for mp in range(16):
                ps1 = bigps()
                for n2 in range(2):
                    s = slice(n2 * 512, (n2 + 1) * 512)
                    for P in range(2):
                        nc.tensor.matmul(
                            ps1[:, s],
                            w1_view(P, mp),
                            xn8[:, 2 * P : 2 * P + 2, s],
                            start=(P == 0),
                            stop=(P == 1),
                            perf_mode=DRSWI,
                        )
                ps2 = bigps()
                for n2 in range(2):
                    s = slice(n2 * 512, (n2 + 1) * 512)
                    for P in range(2):
                        nc.tensor.matmul(
                            ps2[:, s],
                            w1_view(P, mp + 16),
                            xn8[:, 2 * P : 2 * P + 2, s],
                            start=(P == 0),
                            stop=(P == 1),
                            perf_mode=DRSWI,
                        )
                sg = swp.tile([128, NSP], BF16, tag="sw", name="sw")
                nc.scalar.activation(sg[:], ps1[:], AF.Silu, scale=1.0 / FS)
                nc.vector.tensor_mul(act8[:, mp, :], sg[:], ps2[:])

            if KSTAGE == 6:
                for t in range(CT):
                    ft = attn2[t]
                    nc.scalar.mul(ft[:], act8[:, t, :], 1.0 / FS)
                    nc.sync.dma_start(out_d[t * 128 : (t + 1) * 128, :], ft[:])
                return nc, tc

            # reload x into the attn2 slots (attention result consumed by GN2)
            for t in range(CT):
                nc.sync.dma_start(attn2[t][:], x_d[t * 128 : (t + 1) * 128, :])

            # ---- MLP2 + residual -> out ----
            for m in range(CT):
                ps = bigps()
                for n2 in range(2):
                    s = slice(n2 * 512, (n2 + 1) * 512)
                    for P in range(8):
                        nc.tensor.matmul(
                            ps[:, s],
                            w2_view(P, m),
                            act8[:, 2 * P : 2 * P + 2, s],
                            start=(P == 0),
                            stop=(P == 7),
                            perf_mode=DRSWI,
                        )
                m2t = swp.tile([128, NSP], F32, tag="m2t", name="m2t")
                nc.scalar.mul(m2t[:], ps[:], 1.0 / (FS * FS))
                nc.vector.tensor_tensor(attn2[m][:], m2t[:], attn2[m][:], op=ADD)
                nc.sync.dma_start(out_d[m * 128 : (m + 1) * 128, :], attn2[m][:])

    return nc


def _get_nc():
    key = ("nc", KSTAGE)
    if key not in _cache:
        import concourse.bass  # noqa: F401  ensure importable before build
        from concourse import mybir

        res = _build_nc()
        nc = res[0] if isinstance(res, tuple) else res
        _split_multi_waits(nc, mybir, maxw=1)
        _cache[key] = nc
    return _cache[key]


def _prep_weights(inputs):
    bf = ml_dtypes.bfloat16
    f8 = ml_dtypes.float8_e4m3
    f32 = np.float32

    def col4(v):  # (512,) -> (128, 4) with [p, t] = v[128t + p]
        return np.ascontiguousarray(v.reshape(4, 128).T.astype(f32))

    def dr_interleave(wT):
        # wT [K, M] f32 -> [128, (K//256)*(M//128)*256] fp8, scaled by FS,
        # DoubleRowSwInterleave layout: per (k-pair P, m-tile mt) block of
        # 256: flat[:, 2q] = A[:, 127-q], flat[:, 2q+1] = B[:, 127-q].
        K, M = wT.shape
        w = (wT.astype(f32) * FS).astype(f8)
        w3 = w.reshape(K // 128, 128, M)  # [kt, p, m]
        npair, nmt = K // 256, M // 128
        out = np.zeros((128, npair, nmt, 256), f8)
        for P in range(npair):
            A, B = w3[2 * P], w3[2 * P + 1]  # [128, M]
            for mt in range(nmt):
                blk = A[:, mt * 128 : (mt + 1) * 128][:, ::-1]
                out[:, P, mt, 0::2] = blk
                out[:, P, mt, 1::2] = B[:, mt * 128 : (mt + 1) * 128][:, ::-1]
        return np.ascontiguousarray(out.reshape(128, npair * nmt * 256))

    qkv_b = inputs["qkv_b"].astype(f32)
    out_w = inputs["out_w"].astype(f32)
    # fold V bias through the out projection: out = Wo @ (attn + b_v) + b_o
    outb_eff = inputs["out_b"].astype(f32) + out_w @ qkv_b[2 * C : 3 * C]
    sel = np.zeros((C, 32), f32)
    sel[np.arange(C), np.arange(C) // 16] = 1.0
    selbc8 = np.zeros((8, 512), f32)
    for r in range(8):
        selbc8[r, r * 64 : (r + 1) * 64] = 1.0
    selbc8 = selbc8.astype(bf)

    wqkvT = np.ascontiguousarray(inputs["qkv_w"].astype(f32).T)
    wv8 = (wqkvT[:, 2 * C : 3 * C] * FS).astype(f8)  # [512 c, 512 vc]
    wv8 = np.ascontiguousarray(
        wv8.reshape(4, 128, C).transpose(1, 0, 2).reshape(128, 4 * C)
    )
    shared = {
        "wqk8": dr_interleave(wqkvT[:, 0 : 2 * C]),
        "wv8": wv8,
        "qkvb": np.ascontiguousarray(qkv_b[0 : 2 * C].reshape(8, 128).T.astype(f32)),
        "wo8": dr_interleave(np.ascontiguousarray(out_w.T)),
        "outb": col4(outb_eff),
        "g1": col4(inputs["gn1_gamma"].astype(f32)),
        "b1": col4(inputs["gn1_beta"].astype(f32)),
        "g2": col4(inputs["gn2_gamma"].astype(f32)),
        "b2": col4(inputs["gn2_beta"].astype(f32)),
        "w1f8": dr_interleave(np.ascontiguousarray(inputs["mlp1_w"].astype(f32).T)),
        "w2f8": dr_interleave(np.ascontiguousarray(inputs["mlp2_w"].astype(f32).T)),
        "sel": sel,
        "selT": np.ascontiguousarray(sel.T),
        "ident": np.eye(128, dtype=f32).astype(bf),
        "selbc8": selbc8,
    }
    return shared


def kernel(**inputs):
    from concourse.bass_utils import run_bass_kernel_spmd

    nc = _get_nc()
    shared = _prep_weights(inputs)
    x = np.asarray(inputs["x"], dtype=np.float32).reshape(8, C, NSP)
    in_maps = [dict(shared, x=np.ascontiguousarray(x[i])) for i in range(8)]
    res = run_bass_kernel_spmd(nc, in_maps, core_ids=list(range(8))).results
    out = np.stack([res[i]["out"] for i in range(8)], axis=0)
    return out.reshape(8, C, 32, 32).astype(np.float32)
